# revision 9
# baseline (speedup 1.0000x reference)
"""MoE (top-2 of 8 experts, SwiGLU) Trainium2 kernel.

Sharding strategy (expert-parallel, per the hint):
  - Host computes the gate (tiny [T,8] matmul), top-2 routing and softmax
    weights, then performs the "all-to-all" as a host-side gather: tokens
    routed to expert e are packed (padded to a common capacity) and shipped
    to core e together with that expert's weights.
  - Core e computes  y = gate_w * (silu(x @ W1e.T) * (x @ W3e.T)) @ W2e.T
    for its tokens only, in feature-major layout (features on partitions,
    tokens on the free axis) so the SwiGLU intermediate feeds the down-proj
    matmul without any transpose.
  - Host scatter-adds each expert's output rows back into the full output.

All matmuls run as float32r (fp32 storage, single-pass reduced-precision PE
multiply at full rate); accumulation is fp32 in PSUM.
"""

import numpy as np

import concourse.bass as bass
import concourse.mybir as mybir
from concourse import bacc
from concourse import tile
from concourse.bass_utils import run_bass_kernel_spmd

DIM = 1024
HID = 2816
E = 8
TOPK = 2
P = 128
KD = DIM // P  # 8 k-tiles over DIM
KH = HID // P  # 22 k-tiles over HID
F32 = mybir.dt.float32
MM_DT = mybir.dt.float32r  # matmul operand dtype (full-rate for N>=256)
TOK_TILE = 512  # PSUM bank holds 512 fp32

# Test hooks: when TRACE is set (by test.py), the SPMD launch captures an
# NTFF profile and the BassKernelResults lands in LAST_RESULTS.
TRACE = False
LAST_RESULTS = None

_nc_cache: dict = {}


def _tok_slices(cap):
    out = []
    t0 = 0
    while t0 < cap:
        tn = min(TOK_TILE, cap - t0)
        out.append((t0, tn))
        t0 += tn
    return out


def _build_nc(cap):
    """Build the per-core Bass program for capacity `cap` tokens."""
    nc = bacc.Bacc(
        "TRN2",
        target_bir_lowering=False,
        debug=False,
        enable_asserts=False,
        num_devices=E,
    )

    # DRAM I/O (shapes are the host-packed layouts; see kernel() below).
    xt_d = nc.dram_tensor("xp", [KD, P, cap], MM_DT, kind="ExternalInput").ap()
    w1_d = nc.dram_tensor("w1p", [KH, P, KD * P], MM_DT, kind="ExternalInput").ap()
    w3_d = nc.dram_tensor("w3p", [KH, P, KD * P], MM_DT, kind="ExternalInput").ap()
    w2_d = nc.dram_tensor("w2p", [KD, P, KH * P], MM_DT, kind="ExternalInput").ap()
    gw_d = nc.dram_tensor("gwp", [P, cap], F32, kind="ExternalInput").ap()
    yt_d = nc.dram_tensor("yt", [KD, P, cap], F32, kind="ExternalOutput").ap()

    slices = _tok_slices(cap)
    # Group token slices into "blocks" issued per k-tile so that narrow
    # (remainder) matmuls interleave with 512-wide ones and their LDWEIGHTS
    # hide behind matmul streaming. [(s0, s2), (s1)] for the 512/512/128 case.
    if len(slices) >= 3:
        blocks = [[slices[0], slices[2]], [slices[1]]] + [[s] for s in slices[3:]]
    else:
        blocks = [[s] for s in slices]

    with tile.TileContext(nc) as tc:
        with (
            tc.tile_pool(name="xpool", bufs=1) as xpool,
            tc.tile_pool(name="wload", bufs=3) as wload,
            tc.tile_pool(name="w2load", bufs=2) as w2load,
            tc.tile_pool(name="gpool", bufs=1) as gpool,
            tc.tile_pool(name="spool", bufs=2) as spool,
            tc.tile_pool(name="ypool", bufs=3) as ypool,
            tc.tile_pool(name="psA", bufs=1, space="PSUM") as psApool,
            tc.tile_pool(name="psB", bufs=1, space="PSUM") as psBpool,
        ):
            # PE pre-warm: dummy matmuls on a zeroed tile keep the PE busy
            # through the HAM activity window while input DMAs stream.
            t_warm = xpool.tile([P, TOK_TILE], mybir.dt.bfloat16, tag="warm")
            nc.vector.memset(t_warm, 0.0)
            ps_warm = psApool.tile([P, TOK_TILE], F32, tag=f"ps{slices[0][0]}")
            for _ in range(12):
                nc.tensor.matmul(
                    ps_warm, lhsT=t_warm[:, :P], rhs=t_warm, start=True, stop=True
                )

            # Resident activations: [128, KD*cap]; k-th k-tile at cols [k*cap,(k+1)*cap)
            # Split into per-k tiles so matmuls only wait on their own chunk.
            t_xk = []
            for k in range(KD):
                t = xpool.tile([P, cap], MM_DT, tag=f"x{k}")
                nc.sync.dma_start(out=t, in_=xt_d[k])
                t_xk.append(t)
            # Gate weights, pre-broadcast on host to [128, cap]
            t_gw = xpool.tile([P, cap], F32, tag="gw")
            nc.sync.dma_start(out=t_gw, in_=gw_d)

            # ---- Stage 1: G[h, t] = silu(h1) * h3, feature-major ----
            g_tiles = []
            for i in range(KH):
                t_w1 = wload.tile([P, KD * P], MM_DT, tag="w1")
                nc.gpsimd.dma_start(out=t_w1, in_=w1_d[i])
                t_w3 = wload.tile([P, KD * P], MM_DT, tag="w3")
                nc.gpsimd.dma_start(out=t_w3, in_=w3_d[i])

                t_g = gpool.tile([P, cap], MM_DT, tag=f"g{i}")
                g_tiles.append(t_g)

                for t_w, pspool, ps_list in (
                    (t_w1, psApool, []),
                    (t_w3, psBpool, []),
                ):
                    for blk in blocks:
                        ps_tiles = [
                            pspool.tile(
                                [P, TOK_TILE], F32, tag=f"ps{t0}",
                                name=f"s1ps_{i}_{t0}",
                            )
                            for t0, _ in blk
                        ]
                        ps_list.append((blk, ps_tiles))
                        for k in range(KD):
                            for (t0, tn), ps in zip(blk, ps_tiles):
                                nc.tensor.matmul(
                                    ps[:, :tn],
                                    lhsT=t_w[:, k * P : (k + 1) * P],
                                    rhs=t_xk[k][:, t0 : t0 + tn],
                                    start=(k == 0),
                                    stop=(k == KD - 1),
                                )
                    if t_w is t_w1:
                        ps1_list = ps_list
                    else:
                        ps3_list = ps_list

                # silu(h1) * h3 per token slice
                for (blk, ps1s), (_, ps3s) in zip(ps1_list, ps3_list):
                    for (t0, tn), ps1, ps3 in zip(blk, ps1s, ps3s):
                        t_sg = spool.tile([P, TOK_TILE], F32, tag="sig")
                        nc.scalar.activation(
                            t_sg[:, :tn],
                            ps1[:, :tn],
                            mybir.ActivationFunctionType.Sigmoid,
                        )
                        t_s = spool.tile([P, TOK_TILE], F32, tag="silu")
                        nc.vector.tensor_mul(t_s[:, :tn], t_sg[:, :tn], ps1[:, :tn])
                        nc.vector.tensor_mul(
                            t_g[:, t0 : t0 + tn], t_s[:, :tn], ps3[:, :tn]
                        )

            # ---- Stage 2: Y[d, t] = gw[t] * sum_h W2T[h, d] * G[h, t] ----
            for dt_i in range(KD):
                t_w2 = w2load.tile([P, KH * P], MM_DT, tag="w2")
                nc.gpsimd.dma_start(out=t_w2, in_=w2_d[dt_i])
                for blk in blocks:
                    ps_tiles = [
                        psApool.tile(
                            [P, TOK_TILE], F32, tag=f"ps{t0}",
                            name=f"s2ps_{dt_i}_{t0}",
                        )
                        for t0, _ in blk
                    ]
                    for i in range(KH):
                        for (t0, tn), psy in zip(blk, ps_tiles):
                            nc.tensor.matmul(
                                psy[:, :tn],
                                lhsT=t_w2[:, i * P : (i + 1) * P],
                                rhs=g_tiles[i][:, t0 : t0 + tn],
                                start=(i == 0),
                                stop=(i == KH - 1),
                            )
                    for (t0, tn), psy in zip(blk, ps_tiles):
                        t_y = ypool.tile([P, TOK_TILE], F32, tag="y")
                        nc.vector.tensor_mul(
                            t_y[:, :tn], psy[:, :tn], t_gw[:, t0 : t0 + tn]
                        )
                        nc.sync.dma_start(
                            out=yt_d[dt_i][:, t0 : t0 + tn], in_=t_y[:, :tn]
                        )

    nc.compile()
    return nc


def _route(xt, Wg):
    """Top-2 routing identical to the reference (argmax twice + softmax)."""
    scores = xt @ Wg.T  # [T, E] fp32
    top1 = np.argmax(scores, axis=1)
    v1 = scores[np.arange(scores.shape[0]), top1]
    masked = scores.copy()
    masked[np.arange(scores.shape[0]), top1] = -np.inf
    top2 = np.argmax(masked, axis=1)
    v2 = masked[np.arange(scores.shape[0]), top2]
    # softmax over [v1, v2] in fp32 (v1 >= v2)
    e2 = np.exp((v2 - v1).astype(np.float32))
    w1 = (1.0 / (1.0 + e2)).astype(np.float32)
    w2 = (e2 / (1.0 + e2)).astype(np.float32)
    return top1, top2, w1, w2


def kernel(x, Wg, W1, W3, W2):
    x = np.asarray(x, dtype=np.float32)
    Wg = np.asarray(Wg, dtype=np.float32)
    W1 = np.asarray(W1, dtype=np.float32)
    W3 = np.asarray(W3, dtype=np.float32)
    W2 = np.asarray(W2, dtype=np.float32)

    Bsz, Ssz, _ = x.shape
    T = Bsz * Ssz
    xt = x.reshape(T, DIM)

    top1, top2, wt1, wt2 = _route(xt, Wg)

    idx_lists = []
    gw_lists = []
    for e in range(E):
        m1 = np.nonzero(top1 == e)[0]
        m2 = np.nonzero(top2 == e)[0]
        idx_lists.append(np.concatenate([m1, m2]))
        gw_lists.append(np.concatenate([wt1[m1], wt2[m2]]))

    max_cnt = max(len(ix) for ix in idx_lists)
    cap = max(P, ((max_cnt + P - 1) // P) * P)

    if cap not in _nc_cache:
        _nc_cache[cap] = _build_nc(cap)
    nc = _nc_cache[cap]

    in_maps = []
    for e in range(E):
        ix = idx_lists[e]
        n = len(ix)
        # tokens, feature-major, padded: [KD, P, cap]
        xp = np.zeros((DIM, cap), dtype=np.float32)
        xp[:, :n] = xt[ix].T
        # gate weights broadcast across partitions: [P, cap]
        gw = np.zeros((cap,), dtype=np.float32)
        gw[:n] = gw_lists[e]
        gwp = np.ascontiguousarray(np.broadcast_to(gw, (P, cap)))
        # weights packed so each DMA'd tile is contiguous:
        # w1p[i, p, k, c] = W1T[k*P+p, i*P+c] = W1[e, i*P+c, k*P+p]
        w1p = np.ascontiguousarray(
            W1[e].reshape(KH, P, KD, P).transpose(0, 3, 2, 1)
        )
        w3p = np.ascontiguousarray(
            W3[e].reshape(KH, P, KD, P).transpose(0, 3, 2, 1)
        )
        # w2p[dt, p, i, c] = W2T[i*P+p, dt*P+c] = W2[e, dt*P+c, i*P+p]
        w2p = np.ascontiguousarray(
            W2[e].reshape(KD, P, KH, P).transpose(0, 3, 2, 1)
        )
        in_maps.append(
            {
                "xp": np.ascontiguousarray(xp.reshape(KD, P, cap)),
                "w1p": w1p.reshape(KH, P, KD * P),
                "w3p": w3p.reshape(KH, P, KD * P),
                "w2p": w2p.reshape(KD, P, KH * P),
                "gwp": gwp,
            }
        )

    res = run_bass_kernel_spmd(nc, in_maps, list(range(E)), trace=TRACE)
    global LAST_RESULTS
    LAST_RESULTS = res

    out = np.zeros((T, DIM), dtype=np.float32)
    for e in range(E):
        ix = idx_lists[e]
        n = len(ix)
        if n == 0:
            continue
        yt = res.results[e]["yt"].reshape(DIM, -1)  # [DIM, cap]
        out[ix] += yt[:, :n].T
    return out.reshape(Bsz, Ssz, DIM)
